# revision 1
# baseline (speedup 1.0000x reference)
"""Two-layer LSTM (H=51) over [B=4096, T=256] on 8 NeuronCores.

Strategy: data-parallel over batch (512 per core). Per core, a skewed
software pipeline over T+2 phases: phase q computes layer-1 of step q,
layer-2 of step q-1, and the linear head of step q-2.

All sigmoids are rewritten as tanh (sigma(z) = (tanh(z/2)+1)/2) with the
1/2 folded into host-precomputed weights, and states stored doubled
(ht = 2h, ct = 2c), so each phase needs only two ACT instructions
(one tanh over all eight gate matmul outputs, one tanh(c/2)) and four
fused scalar_tensor_tensor DVE instructions.
"""

import numpy as np

H = 51
T_FULL = 256
B_FULL = 4096
N_CORES = 8

# Stk partition layout (stacked matmul rhs). Compute-engine writes must
# start at a 32-aligned partition, so the states live at rows 0..114
# (matching the gate-row space) and the constant rows sit above them:
#   rows 0..50   : ht1 (= 2*h1)
#   rows 51..63  : junk (zero, weighted by zero)
#   rows 64..114 : ht2 (= 2*h2)
#   row 115      : ones (bias row, DMA-initialized)
#   row 116      : x_t (DMA per step)
ROW_H1 = 0
ROW_JUNK = 51
ROW_H2 = 64
ROW_ONES = 115
ROW_X = 116
K_STK = 117
# gate-row space of the elementwise ops: rows 0..50 layer1, 51..63 junk,
# 64..114 layer2
GP = 115


MW = GP  # matmul output width (zero-padded gate lhsT columns)


def _build_weights(W_ih1, W_hh1, b_ih1, b_hh1, W_ih2, W_hh2, b_ih2, b_hh2,
                   W_lin, b_lin):
    """Host-side packing of lhsT weight tiles.

    Returns WG [K_STK, 8*MW + 1] float32. Eight gate lhsTs of width MW=115
    (layer1 i,f,o,g then layer2 i,f,o,g), zero-padded so that both layers'
    matmuls write the full [115, B] PSUM region at partition base 0 (f32r
    matmuls require base 0): layer-1 weights occupy output rows 0..50 and
    clear the rest with zero columns (start=True); layer-2 weights occupy
    rows 64..114 and accumulate (start=False).
    Column 8*MW rows 64..115: [0.5*W_lin; b_lin] for the out head
    (lhsT partitions must match its rhs Stk[64:116] = [ht2; ones]).
    Gate scaling: sigma-gates (i,f,o) rows scaled by 0.5 (tanh(z/2) trick);
    h inputs scaled by 0.5 (states stored doubled).
    """
    b1 = (b_ih1 + b_hh1).astype(np.float64)
    b2 = (b_ih2 + b_hh2).astype(np.float64)
    # reference gate order in the stacked 4H rows: i, f, g, o
    idx = {"i": np.arange(0, H), "f": np.arange(H, 2 * H),
           "g": np.arange(2 * H, 3 * H), "o": np.arange(3 * H, 4 * H)}
    # our bank order: i, g first (v = (ti+1)*tg depends only on these, so
    # the first of the two split tanh ops unblocks it), then f, o
    order = ["i", "g", "f", "o"]
    WG = np.zeros((K_STK, 8 * MW + 1), dtype=np.float64)
    for xi, gate in enumerate(order):
        r = idx[gate]
        s = 0.5 if gate in ("i", "f", "o") else 1.0
        col = slice(xi * MW, xi * MW + H)  # output rows 0..50 (noqa)
        # layer 1: z1 = W_ih1 @ x + b1 + W_hh1 @ h1
        WG[ROW_ONES, col] = s * b1[r]
        WG[ROW_H1:ROW_H1 + H, col] = s * 0.5 * W_hh1[r, :].T
        WG[ROW_X, col] = s * W_ih1[r, 0]
    for xi, gate in enumerate(order):
        r = idx[gate]
        s = 0.5 if gate in ("i", "f", "o") else 1.0
        col = slice((4 + xi) * MW + ROW_H2, (4 + xi) * MW + ROW_H2 + H)
        # layer 2: z2 = W_ih2 @ h1 + b2 + W_hh2 @ h2 (output rows 64..114)
        WG[ROW_ONES, col] = s * b2[r]
        WG[ROW_H1:ROW_H1 + H, col] = s * 0.5 * W_ih2[r, :].T
        WG[ROW_H2:ROW_H2 + H, col] = s * 0.5 * W_hh2[r, :].T
    # out head: lhsT must sit at the same partitions as its rhs Stk[64:116]
    # (= [ht2 (51); ones]), so W_lin goes at rows 64..114 and b_lin at 115.
    WG[ROW_H2:ROW_H2 + H, 8 * MW] = 0.5 * W_lin[0, :]
    WG[ROW_ONES, 8 * MW] = float(np.asarray(b_lin).reshape(-1)[0])
    return np.ascontiguousarray(WG).astype(np.float32)


def build_core_kernel(T, B, groups=2, use_f32r=True):
    """Build the per-core Bass kernel. Inputs: xT [T, B], WG [K_STK, 409].
    Output: outT [T, B] (linear head, WITHOUT b_lin)."""
    import concourse.bacc as bacc
    import concourse.mybir as mybir
    from concourse.tile import TileContext

    fp = mybir.dt.float32
    # Matmul operands are float32r (full-rate fp32 path). The verifier
    # requires f32r typing end-to-end, so the state/weight/x tensors are
    # natively f32r; h-tilde is only ever consumed by matmuls, so rounding
    # at the DVE write loses nothing vs rounding at the PE read.
    fpr = mybir.dt.float32r if use_f32r else fp
    Bg = B // groups

    nc = bacc.Bacc("TRN2", target_bir_lowering=False, debug=False)
    # xT row 0 is a host-prepended row of ones (feeds the bias row of Stk);
    # rows 1..T are input.T
    xT = nc.dram_tensor("xT", [T + 1, B], fpr, kind="ExternalInput")
    WG = nc.dram_tensor("WG", [K_STK, 8 * MW + 1], fpr, kind="ExternalInput")
    out_bt = nc.dram_tensor("out_bt", [B, T], fp, kind="ExternalOutput")

    C = min(128, T)  # output columns buffered in PSUM between flushes
    assert T % C == 0
    assert (B // groups) % 128 == 0, "batch per group must be a multiple of 128"

    with TileContext(nc) as tc:
        with (
            tc.tile_pool(name="persist", bufs=1) as persist,
            tc.tile_pool(name="gpsum", bufs=1, space="PSUM") as gpsum,
            tc.tile_pool(name="opsum", bufs=1, space="PSUM") as opsum,
            tc.tile_pool(name="temps", bufs=3) as temps,
            tc.tile_pool(name="ostage", bufs=2) as ostage,
        ):
            wg = persist.tile([K_STK, 8 * MW + 1], fpr)
            nc.sync.dma_start(out=wg, in_=WG[:, :])

            nchunk = Bg // 128
            stks, cts, gps, pos = [], [], [], []
            for g in range(groups):
                stk = persist.tile([K_STK, Bg], fpr, tag=f"stk{g}")
                ct = persist.tile([GP, Bg], fp, tag=f"ct{g}")
                gp = gpsum.tile([GP, 4 * Bg], fp, tag=f"gp{g}")
                # DVE memset can't target f32r directly; write zero bits
                # through an f32 view (0.0 is exact in f32r).
                nc.vector.memset(stk[:, :].bitcast(fp), 0.0)
                nc.sync.dma_start(out=stk[ROW_ONES:ROW_ONES + 1, :],
                                  in_=xT[0:1, g * Bg:(g + 1) * Bg])
                nc.vector.memset(ct[:, :], 0.0)
                stks.append(stk)
                cts.append(ct)
                gps.append(gp)
                pos.append(opsum.tile([128, nchunk * C], fp, tag=f"po{g}",
                                      name=f"po{g}"))

            add = mybir.AluOpType.add
            mult = mybir.AluOpType.mult
            tanh = mybir.ActivationFunctionType.Tanh

            for q in range(T + 2):
                for g in range(groups):
                    stk, ct, gp = stks[g], cts[g], gps[g]
                    cols = slice(g * Bg, (g + 1) * Bg)
                    # ---- x load for step q (xT is offset by the ones row)
                    if q < T:
                        nc.sync.dma_start(out=stk[ROW_X:ROW_X + 1, :],
                                          in_=xT[q + 1:q + 2, cols])
                    # ---- gate matmuls: both layers write the full [115, Bg]
                    # region at base 0 (f32r needs base 0); layer-1's
                    # zero-padded lhsT clears rows 51..114, layer-2
                    # accumulates into rows 64..114.
                    rhs = stk[0:K_STK, :]
                    l1 = q < T
                    l2 = 1 <= q <= T
                    for xi in range(4):
                        if l1:
                            nc.tensor.matmul(
                                gp[0:GP, xi * Bg:(xi + 1) * Bg],
                                wg[0:K_STK, xi * MW:xi * MW + MW],
                                rhs, start=True, stop=not l2)
                        if l2:
                            nc.tensor.matmul(
                                gp[0:GP, xi * Bg:(xi + 1) * Bg],
                                wg[0:K_STK, (4 + xi) * MW:(5 + xi) * MW],
                                rhs, start=not l1, stop=True)
                    # ---- out head for step t = q-2: out[:, t] column
                    if q >= 2:
                        t = q - 2
                        tc_col = t % C
                        for k in range(nchunk):
                            # f32r rejects N=1 matmuls; run the tiny out
                            # head in plain f32 via bitcast views.
                            nc.tensor.matmul(
                                pos[g][:, k * C + tc_col:k * C + tc_col + 1],
                                stk[64:116, k * 128:(k + 1) * 128].bitcast(fp),
                                wg[64:116, 8 * MW:8 * MW + 1].bitcast(fp),
                                start=True, stop=True)
                        if tc_col == C - 1:  # flush epoch
                            t0 = t - (C - 1)
                            for k in range(nchunk):
                                st = ostage.tile([128, C], fp, tag=f"os{g}_{k}")
                                nc.scalar.copy(st, pos[g][:, k * C:(k + 1) * C])
                                row0 = g * Bg + k * 128
                                nc.sync.dma_start(
                                    out=out_bt[row0:row0 + 128, t0:t0 + C],
                                    in_=st)
                    # ---- elementwise chain (banks: 0=i, 1=g, 2=f, 3=o).
                    # tanh is split in two so v = (ti+1)*tg can start after
                    # only the i,g matmuls; f,o matmuls overlap the first
                    # tanh on the PE.
                    if q <= T:
                        tg_t = temps.tile([GP, 4 * Bg], fp, tag=f"tg{g}")
                        nc.scalar.activation(tg_t[:, 0:2 * Bg],
                                             gp[0:GP, 0:2 * Bg], tanh)
                        nc.scalar.activation(tg_t[:, 2 * Bg:4 * Bg],
                                             gp[0:GP, 2 * Bg:4 * Bg], tanh)
                        ti = tg_t[:, 0 * Bg:1 * Bg]
                        tg = tg_t[:, 1 * Bg:2 * Bg]
                        tf = tg_t[:, 2 * Bg:3 * Bg]
                        to = tg_t[:, 3 * Bg:4 * Bg]
                        u = temps.tile([GP, Bg], fp, tag=f"u{g}")
                        v = temps.tile([GP, Bg], fp, tag=f"v{g}")
                        tcl = temps.tile([GP, Bg], fp, tag=f"tc{g}")
                        # v = (ti+1)*tg ; u = (tf+1)*ct ; ct = 0.5*u + v
                        nc.vector.scalar_tensor_tensor(v, ti, 1.0, tg, add, mult)
                        nc.vector.scalar_tensor_tensor(u, tf, 1.0, ct[:, :], add, mult)
                        nc.vector.scalar_tensor_tensor(ct[:, :], u, 0.5, v, mult, add)
                        # tanh(c) = tanh(0.5*ct); ht = (to+1)*tanh(c)
                        nc.scalar.activation(tcl, ct[:, :], tanh, scale=0.5)
                        nc.vector.scalar_tensor_tensor(
                            stk[ROW_H1:ROW_H1 + GP, :], to, 1.0, tcl, add, mult)
    nc.compile()
    return nc


_NC_CACHE = {}


def _get_nc(T, B, groups, use_f32r):
    key = (T, B, groups, use_f32r)
    if key not in _NC_CACHE:
        _NC_CACHE[key] = build_core_kernel(T, B, groups, use_f32r)
    return _NC_CACHE[key]


def kernel(input, W_ih1, W_hh1, b_ih1, b_hh1, W_ih2, W_hh2, b_ih2, b_hh2,
           W_lin, b_lin, _groups=2, _use_f32r=True):
    from concourse import bass_utils

    input = np.asarray(input, dtype=np.float32)
    B, T = input.shape
    Bc = B // N_CORES
    WG = _build_weights(np.asarray(W_ih1, np.float64), np.asarray(W_hh1, np.float64),
                        np.asarray(b_ih1, np.float64), np.asarray(b_hh1, np.float64),
                        np.asarray(W_ih2, np.float64), np.asarray(W_hh2, np.float64),
                        np.asarray(b_ih2, np.float64), np.asarray(b_hh2, np.float64),
                        np.asarray(W_lin, np.float64), np.asarray(b_lin, np.float64))
    # row 0 = ones (bias row), rows 1..T = input.T
    xT = np.concatenate([np.ones((1, B), np.float32), input.T.astype(np.float32)])
    nc = _get_nc(T, Bc, _groups, _use_f32r)
    in_maps = [
        {"xT": np.ascontiguousarray(xT[:, c * Bc:(c + 1) * Bc]), "WG": WG}
        for c in range(N_CORES)
    ]
    res = bass_utils.run_bass_kernel_spmd(
        nc, in_maps, core_ids=list(range(N_CORES)), trace=False)
    outs = [res.results[c]["out_bt"] for c in range(N_CORES)]  # [Bc, T] each
    out = np.concatenate(outs, axis=0)  # [B, T]
    return out.astype(np.float32)



# revision 45
# speedup vs baseline: 1.2159x; 1.2159x over previous
"""Two-layer LSTM (H=51) over [B=4096, T=256] on 8 NeuronCores.

Data-parallel over batch (512 per core), skewed software pipeline over T+2
phases: phase q computes layer-1 of step q together with layer-2 of step
q-1 (merged into the same matmuls), and the linear head of step q-2.

Per phase (per batch-group of 256):
  - 4 merged gate matmuls: one lhsT carries BOTH layers' weights
    (layer-1 -> output rows 0..50, layer-2 -> rows 64..114), two gate
    banks per PSUM tile (assignment configurable).
  - x is prefetched 4 steps per DMA into 12 rotating rhs partition slots
    (rows 116..127); each of the 12 lhsT variants reads its own x row.
  - sigma(z) = (tanh(z/2)+1)/2 with the 1/2 folded into the weights and
    states stored doubled (ht=2h, ct=2c); elementwise tail:
    u=(tf+1)*ct, v=(ti+1)*tg, ct'=0.5u+v, tcl=tanh(ct'/2), ht=(to+1)*tcl.

Hardware legality notes (bir verifier):
  - Pool/gpsimd instructions cannot access PSUM at all.
  - Other engines may read at most ONE non-scalar input from PSUM.
"""

import numpy as np

H = 51
T_FULL = 256
B_FULL = 4096
N_CORES = 8

ROW_H1 = 0      # rows 0..50: ht1 (= 2*h1)
ROW_H2 = 64     # rows 64..114: ht2 (= 2*h2)  (64: 32-aligned lhsT base for head)
ROW_ONES = 115  # bias row
ROW_X0 = 116    # rows 116..127: 12 x slots (3 rotating DMA banks of 4)
NXSLOT = 12
XBANK = 4       # steps per x-prefetch DMA
K_STK = 128
GP = 115        # gate-row space: 0..50 layer1, 51..63 junk, 64..114 layer2
MW = 115        # matmul output width per bank

N_VAR = NXSLOT * 4          # merged variant banks
L1B = N_VAR                 # 4 layer-1-only banks (phase 0, x slot 0)
L2B = N_VAR + 4             # 4 layer-2-only banks (phase T)
HEADCOL = (N_VAR + 8) * MW  # head column
NCOL = HEADCOL + 1

# schedule/assignment knobs (sweepable):
#   t1/t2: gate pair per PSUM tile; t?_out: tanh dest ("sb" SBUF tile,
#   "ip" in-place PSUM); t?_split: one tanh instr per gate instead of a
#   joint one; engines: "dve" | "pool" (pool requires SBUF operands).
CONFIG = {
    "t1": "ig", "t2": "fo", "t1_out": "sb", "t2_out": "sb",
    "t1_split": False, "t2_split": True,
    "v_eng": "dve", "u_eng": "dve", "ct_eng": "dve", "ht_eng": "pool",
    # f_sig: f-gate is a real Sigmoid (joint [f,o] sigmoid instr);
    # u = sf*ct and ct' = u+v become plain tensor_tensor ops.
    "f_sig": True,
    # ht_eng "split": ht computed in two column-halves in parallel,
    # cols [0:ht_dve_cols] on DVE and the rest on Pool.
    "ht_dve_cols": 176,
}


def _gate_order():
    return list(CONFIG["t1"]) + list(CONFIG["t2"])


def _build_weights(W_ih1, W_hh1, b_ih1, b_hh1, W_ih2, W_hh2, b_ih2, b_hh2,
                   W_lin, b_lin):
    """Host-side packing of lhsT weight banks -> WG [K_STK, NCOL] f32."""
    b1 = (b_ih1 + b_hh1).astype(np.float64)
    b2 = (b_ih2 + b_hh2).astype(np.float64)
    idx = {"i": np.arange(0, H), "f": np.arange(H, 2 * H),
           "g": np.arange(2 * H, 3 * H), "o": np.arange(3 * H, 4 * H)}
    WG = np.zeros((K_STK, NCOL), dtype=np.float64)
    order = _gate_order()

    f_sig = CONFIG["f_sig"]

    def fill_bank(col0, gate, l1, l2, xslot):
        # h is stored UNdoubled; c doubled (ct=2c). i (and f unless
        # f_sig) use the tanh-half-angle trick (s=0.5); g is tanh;
        # o (and f if f_sig) are real Sigmoids.
        r = idx[gate]
        s = 0.5 if (gate == "i" or (gate == "f" and not f_sig)) else 1.0
        if l1:
            c1 = slice(col0, col0 + H)  # output rows 0..50
            WG[ROW_H1:ROW_H1 + H, c1] += s * W_hh1[r, :].T
            WG[ROW_ONES, c1] += s * b1[r]
            WG[ROW_X0 + xslot, c1] += s * W_ih1[r, 0]
        if l2:
            c2 = slice(col0 + ROW_H2, col0 + ROW_H2 + H)  # rows 64..114
            WG[ROW_H1:ROW_H1 + H, c2] += s * W_ih2[r, :].T
            WG[ROW_H2:ROW_H2 + H, c2] += s * W_hh2[r, :].T
            WG[ROW_ONES, c2] += s * b2[r]

    for slot in range(NXSLOT):
        for bi, gate in enumerate(order):
            fill_bank((slot * 4 + bi) * MW, gate, True, True, slot)
    for bi, gate in enumerate(order):
        fill_bank((L1B + bi) * MW, gate, True, False, 0)
        fill_bank((L2B + bi) * MW, gate, False, True, 0)
    WG[ROW_H2:ROW_H2 + H, HEADCOL] = W_lin[0, :]
    WG[ROW_ONES, HEADCOL] = float(np.asarray(b_lin).reshape(-1)[0])
    return np.ascontiguousarray(WG).astype(np.float32)


def build_core_kernel(T, B, groups=2, use_f32r=True):
    """Per-core Bass kernel. Inputs: xT [T+1, B] (row 0 = ones), WG.
    Output: out_bt [B, T]."""
    import concourse.bacc as bacc
    import concourse.mybir as mybir
    from concourse.tile import TileContext

    fp = mybir.dt.float32
    fpr = mybir.dt.float32r if use_f32r else fp
    Bg = B // groups
    assert B % groups == 0 and Bg % 128 == 0

    nc = bacc.Bacc("TRN2", target_bir_lowering=False, debug=False)
    xT = nc.dram_tensor("xT", [T + 1, B], fpr, kind="ExternalInput")
    WG = nc.dram_tensor("WG", [K_STK, NCOL], fpr, kind="ExternalInput")
    out_bt = nc.dram_tensor("out_bt", [B, T], fp, kind="ExternalOutput")

    C = min(128, T)          # head columns buffered in PSUM between flushes
    assert T % C == 0
    nchunk = B // 128
    assert T % XBANK == 0

    add = mybir.AluOpType.add
    mult = mybir.AluOpType.mult
    tanh = mybir.ActivationFunctionType.Tanh
    sigmoid = mybir.ActivationFunctionType.Sigmoid
    f_sig = CONFIG["f_sig"]
    gfunc = {"i": tanh, "g": tanh,
             "f": sigmoid if f_sig else tanh, "o": sigmoid}
    eng = {"dve": nc.vector, "pool": nc.gpsimd}
    v_eng = eng[CONFIG["v_eng"]]
    u_eng = eng[CONFIG["u_eng"]]
    ct_eng = eng[CONFIG["ct_eng"]]
    ht_split = CONFIG["ht_eng"] == "split"
    ht_eng = None if ht_split else eng[CONFIG["ht_eng"]]
    tiles_cfg = [(CONFIG["t1"], CONFIG["t1_out"], CONFIG["t1_split"]),
                 (CONFIG["t2"], CONFIG["t2_out"], CONFIG["t2_split"])]
    order = _gate_order()
    gate_tile = {}
    for tix, (gates, out, split) in enumerate(tiles_cfg):
        for ci, gname in enumerate(gates):
            gate_tile[gname] = (tix, ci)
    sb_gate = {g: tiles_cfg[gate_tile[g][0]][1] == "sb" for g in "igfo"}
    # legality: pool needs SBUF operands and supports only tensor_tensor
    # (no scalar_tensor_tensor); others: <=1 PSUM input per instruction
    assert sb_gate["i"] or sb_gate["g"], "v would read 2 PSUM inputs"
    assert CONFIG["v_eng"] != "pool", "v is stt; unsupported on Pool"
    if CONFIG["u_eng"] == "pool":
        assert f_sig and sb_gate["f"]
    if CONFIG["ct_eng"] == "pool":
        assert f_sig
    if CONFIG["ht_eng"] in ("pool", "split"):
        assert sb_gate["o"]

    with TileContext(nc) as tc:
        with (
            tc.tile_pool(name="persist", bufs=1) as persist,
            tc.tile_pool(name="gpsum", bufs=1, space="PSUM") as gpsum,
            tc.tile_pool(name="opsum", bufs=1, space="PSUM") as opsum,
        ):
            wg = persist.tile([K_STK, NCOL], fpr)
            nc.sync.dma_start(out=wg, in_=WG[:, :])

            stk = persist.tile([K_STK, B], fpr, tag="stk")
            ctt = persist.tile([GP, B], fp, tag="ctt")
            nc.vector.memset(stk[:, :].bitcast(fp), 0.0)
            nc.vector.memset(ctt[:, :], 0.0)
            nc.sync.dma_start(out=stk[ROW_ONES:ROW_ONES + 1, :],
                              in_=xT[0:1, :])
            # preload x windows 0 and 4 (phases 0..7)
            nc.sync.dma_start(out=stk[ROW_X0:ROW_X0 + XBANK, :],
                              in_=xT[1:1 + XBANK, :])
            if T > XBANK:
                nc.sync.dma_start(
                    out=stk[ROW_X0 + XBANK:ROW_X0 + 2 * XBANK, :],
                    in_=xT[1 + XBANK:1 + 2 * XBANK, :])

            gpt = [[gpsum.tile([GP, 2 * Bg], fp, tag=f"gp{t}{g}",
                               name=f"gp{t}{g}") for t in range(2)]
                   for g in range(groups)]
            sbt = [[persist.tile([GP, 2 * Bg], fp, tag=f"sb{t}{g}",
                                 name=f"sb{t}{g}")
                    if tiles_cfg[t][1] == "sb" else None for t in range(2)]
                   for g in range(groups)]
            pos = opsum.tile([128, nchunk * C], fp, tag="pos")
            us = [persist.tile([GP, Bg], fp, tag=f"u{g}", name=f"u{g}")
                  for g in range(groups)]
            vs = [persist.tile([GP, Bg], fp, tag=f"v{g}", name=f"v{g}")
                  for g in range(groups)]
            tcls = [persist.tile([GP, Bg], fp, tag=f"tcl{g}", name=f"tcl{g}")
                    for g in range(groups)]
            ost = persist.tile([128, nchunk * C], fp, tag="ost", name="ost")

            def gate_res(g, gname):
                tix, ci = gate_tile[gname]
                src = sbt[g][tix] if tiles_cfg[tix][1] == "sb" else gpt[g][tix]
                return src[:, ci * Bg:(ci + 1) * Bg]

            for q in range(T + 2):
                # ---- x prefetch: window w = q+6 (covers phases w..w+3),
                # rotating over 3 banks of 4 partition rows
                w = q + 6
                if w % XBANK == 0 and w < T:
                    bank = (w // XBANK) % (NXSLOT // XBANK)
                    nc.sync.dma_start(
                        out=stk[ROW_X0 + bank * XBANK:
                                ROW_X0 + (bank + 1) * XBANK, :],
                        in_=xT[1 + w:1 + w + XBANK, :])

                for g in range(groups):
                    cols = slice(g * Bg, (g + 1) * Bg)
                    if q > T:
                        continue
                    if q == 0:
                        vb = L1B
                    elif q == T:
                        vb = L2B
                    else:
                        vb = (q % NXSLOT) * 4
                    rhs = stk[0:K_STK, cols]
                    for bi in range(4):
                        tix, ci = gate_tile[order[bi]]
                        nc.tensor.matmul(
                            gpt[g][tix][0:GP, ci * Bg:(ci + 1) * Bg],
                            wg[0:K_STK, (vb + bi) * MW:(vb + bi + 1) * MW],
                            rhs, start=True, stop=True)
                    # tanh per tile (joint or split), v as soon as i,g are
                    # done, u as soon as f is done
                    done = set()
                    emitted_v = emitted_u = False
                    for tix, (gates, out, split) in enumerate(tiles_cfg):
                        src = gpt[g][tix]
                        dst = sbt[g][tix] if out == "sb" else src
                        if split:
                            for ci, gname in enumerate(gates):
                                nc.scalar.activation(
                                    dst[:, ci * Bg:(ci + 1) * Bg],
                                    src[:, ci * Bg:(ci + 1) * Bg],
                                    gfunc[gname])
                                done.add(gname)
                                emitted_v, emitted_u = _maybe_uv(
                                    nc, g, cols, done, emitted_v, emitted_u,
                                    v_eng, u_eng, vs, us, ctt, gate_res,
                                    add, mult, f_sig)
                        else:
                            # one func per instruction
                            assert len({gfunc[x] for x in gates}) == 1
                            nc.scalar.activation(dst[:, :], src[:, :],
                                                 gfunc[gates[0]])
                            done.update(gates)
                            emitted_v, emitted_u = _maybe_uv(
                                nc, g, cols, done, emitted_v, emitted_u,
                                v_eng, u_eng, vs, us, ctt, gate_res,
                                add, mult, f_sig)
                    if f_sig:
                        # ct' = u + v  (u = sf*ct)
                        ct_eng.tensor_tensor(ctt[:, cols], us[g], vs[g], add)
                    else:
                        # ct' = 0.5*u + v  (u = (tf+1)*ct)
                        ct_eng.scalar_tensor_tensor(
                            ctt[:, cols], us[g], 0.5, vs[g], mult, add)

                # ---- head for step t = q-2 (reads stk h2 written last phase)
                if q >= 2:
                    t = q - 2
                    tcc = t % C
                    for k in range(nchunk):
                        nc.tensor.matmul(
                            pos[:, k * C + tcc:k * C + tcc + 1],
                            stk[64:116, k * 128:(k + 1) * 128].bitcast(fp),
                            wg[64:116, HEADCOL:HEADCOL + 1].bitcast(fp),
                            start=True, stop=True)
                    if tcc == C - 1:
                        t0 = t - (C - 1)
                        for k in range(nchunk):
                            # stage PSUM->SBUF on DVE (DMA and Pool can't
                            # read PSUM)
                            nc.vector.tensor_scalar(
                                ost[:, k * C:(k + 1) * C],
                                pos[:, k * C:(k + 1) * C], 0.0, None, add)
                            nc.sync.dma_start(
                                out=out_bt[k * 128:(k + 1) * 128, t0:t0 + C],
                                in_=ost[:, k * C:(k + 1) * C])

                # ---- tails: alternate tcl/ht per group
                if q <= T:
                    for g in range(groups):
                        cols = slice(g * Bg, (g + 1) * Bg)
                        # tcl = tanh(ct/2) = tanh(c)
                        nc.scalar.activation(tcls[g], ctt[:, cols], tanh,
                                             scale=0.5)
                        # ht = so * tcl  (h stored undoubled; so = sigmoid)
                        so = gate_res(g, "o")
                        if ht_split:
                            hd = CONFIG["ht_dve_cols"]
                            c0 = g * Bg
                            nc.vector.tensor_tensor(
                                stk[ROW_H1:ROW_H1 + GP, c0:c0 + hd],
                                so[:, 0:hd], tcls[g][:, 0:hd], mult)
                            nc.gpsimd.tensor_tensor(
                                stk[ROW_H1:ROW_H1 + GP, c0 + hd:c0 + Bg],
                                so[:, hd:Bg], tcls[g][:, hd:Bg], mult)
                        else:
                            ht_eng.tensor_tensor(
                                stk[ROW_H1:ROW_H1 + GP, cols],
                                so, tcls[g], mult)
    nc.compile()
    return nc


def _maybe_uv(nc, g, cols, done, emitted_v, emitted_u, v_eng, u_eng,
              vs, us, ctt, gate_res, add, mult, f_sig):
    if not emitted_v and "i" in done and "g" in done:
        # v = (ti+1)*tg  (= 2*sigma_i*gtilde)
        v_eng.scalar_tensor_tensor(
            vs[g], gate_res(g, "i"), 1.0, gate_res(g, "g"), add, mult)
        emitted_v = True
    if not emitted_u and "f" in done:
        if f_sig:
            # u = sf * ct  (= 2*sigma_f*c)
            u_eng.tensor_tensor(us[g], gate_res(g, "f"), ctt[:, cols], mult)
        else:
            # u = (tf+1)*ct
            u_eng.scalar_tensor_tensor(
                us[g], gate_res(g, "f"), 1.0, ctt[:, cols], add, mult)
        emitted_u = True
    return emitted_v, emitted_u


_NC_CACHE = {}


def _get_nc(T, B, groups=2, use_f32r=True):
    key = (T, B, groups, use_f32r, tuple(sorted(CONFIG.items())))
    if key not in _NC_CACHE:
        _NC_CACHE[key] = build_core_kernel(T, B, groups, use_f32r)
    return _NC_CACHE[key]


def kernel(input, W_ih1, W_hh1, b_ih1, b_hh1, W_ih2, W_hh2, b_ih2, b_hh2,
           W_lin, b_lin, _groups=2, _use_f32r=True):
    from concourse import bass_utils

    input = np.asarray(input, dtype=np.float32)
    B, T = input.shape
    Bc = B // N_CORES
    WG = _build_weights(
        np.asarray(W_ih1, np.float64), np.asarray(W_hh1, np.float64),
        np.asarray(b_ih1, np.float64), np.asarray(b_hh1, np.float64),
        np.asarray(W_ih2, np.float64), np.asarray(W_hh2, np.float64),
        np.asarray(b_ih2, np.float64), np.asarray(b_hh2, np.float64),
        np.asarray(W_lin, np.float64), np.asarray(b_lin, np.float64))
    xT = np.concatenate([np.ones((1, B), np.float32),
                         input.T.astype(np.float32)])
    nc = _get_nc(T, Bc, _groups, _use_f32r)
    in_maps = [
        {"xT": np.ascontiguousarray(xT[:, c * Bc:(c + 1) * Bc]), "WG": WG}
        for c in range(N_CORES)
    ]
    res = bass_utils.run_bass_kernel_spmd(
        nc, in_maps, core_ids=list(range(N_CORES)), trace=False)
    outs = [res.results[c]["out_bt"] for c in range(N_CORES)]
    out = np.concatenate(outs, axis=0)
    return out.astype(np.float32)


# revision 48
# speedup vs baseline: 1.2263x; 1.0085x over previous
"""Two-layer LSTM (H=51) over [B=4096, T=256] on 8 NeuronCores.

Data-parallel over batch (512 per core), skewed software pipeline over T+2
phases: phase q computes layer-1 of step q together with layer-2 of step
q-1 (merged into the same matmuls), and the linear head of step q-2.

Per phase (per batch-group of 256):
  - 4 merged gate matmuls: one lhsT carries BOTH layers' weights
    (layer-1 -> output rows 0..50, layer-2 -> rows 64..114), two gate
    banks per PSUM tile (assignment configurable).
  - x is prefetched 4 steps per DMA into 12 rotating rhs partition slots
    (rows 116..127); each of the 12 lhsT variants reads its own x row.
  - sigma(z) = (tanh(z/2)+1)/2 with the 1/2 folded into the weights and
    states stored doubled (ht=2h, ct=2c); elementwise tail:
    u=(tf+1)*ct, v=(ti+1)*tg, ct'=0.5u+v, tcl=tanh(ct'/2), ht=(to+1)*tcl.

Hardware legality notes (bir verifier):
  - Pool/gpsimd instructions cannot access PSUM at all.
  - Other engines may read at most ONE non-scalar input from PSUM.
"""

import numpy as np

H = 51
T_FULL = 256
B_FULL = 4096
N_CORES = 8

ROW_H1 = 0      # rows 0..50: ht1 (= 2*h1)
ROW_H2 = 64     # rows 64..114: ht2 (= 2*h2)  (64: 32-aligned lhsT base for head)
ROW_ONES = 115  # bias row
ROW_X0 = 116    # rows 116..127: 12 x slots (3 rotating DMA banks of 4)
NXSLOT = 12
XBANK = 4       # steps per x-prefetch DMA
K_STK = 128
GP = 115        # gate-row space: 0..50 layer1, 51..63 junk, 64..114 layer2
MW = 115        # matmul output width per bank

N_VAR = NXSLOT * 4          # merged variant banks
L1B = N_VAR                 # 4 layer-1-only banks (phase 0, x slot 0)
L2B = N_VAR + 4             # 4 layer-2-only banks (phase T)
HEADCOL = (N_VAR + 8) * MW  # head column
NCOL = HEADCOL + 1

# schedule/assignment knobs (sweepable):
#   t1/t2: gate pair per PSUM tile; t?_out: tanh dest ("sb" SBUF tile,
#   "ip" in-place PSUM); t?_split: one tanh instr per gate instead of a
#   joint one; engines: "dve" | "pool" (pool requires SBUF operands).
CONFIG = {
    "t1": "ig", "t2": "fo", "t1_out": "sb", "t2_out": "sb",
    "t1_split": False, "t2_split": True,
    "v_eng": "dve", "u_eng": "dve", "ct_eng": "dve", "ht_eng": "dve",
    # f_sig: f-gate is a real Sigmoid (joint [f,o] sigmoid instr);
    # u = sf*ct and ct' = u+v become plain tensor_tensor ops.
    "f_sig": True,
    # ht_eng "split": ht computed in two column-halves in parallel,
    # cols [0:ht_dve_cols] on DVE and the rest on Pool.
    "ht_dve_cols": 176,
    # issue tcl/ht inside each group's block instead of a shared tail loop
    "tails_inline": True,
}


def _gate_order():
    return list(CONFIG["t1"]) + list(CONFIG["t2"])


def _build_weights(W_ih1, W_hh1, b_ih1, b_hh1, W_ih2, W_hh2, b_ih2, b_hh2,
                   W_lin, b_lin):
    """Host-side packing of lhsT weight banks -> WG [K_STK, NCOL] f32."""
    b1 = (b_ih1 + b_hh1).astype(np.float64)
    b2 = (b_ih2 + b_hh2).astype(np.float64)
    idx = {"i": np.arange(0, H), "f": np.arange(H, 2 * H),
           "g": np.arange(2 * H, 3 * H), "o": np.arange(3 * H, 4 * H)}
    WG = np.zeros((K_STK, NCOL), dtype=np.float64)
    order = _gate_order()

    f_sig = CONFIG["f_sig"]

    def fill_bank(col0, gate, l1, l2, xslot):
        # h is stored UNdoubled; c doubled (ct=2c). i (and f unless
        # f_sig) use the tanh-half-angle trick (s=0.5); g is tanh;
        # o (and f if f_sig) are real Sigmoids.
        r = idx[gate]
        s = 0.5 if (gate == "i" or (gate == "f" and not f_sig)) else 1.0
        if l1:
            c1 = slice(col0, col0 + H)  # output rows 0..50
            WG[ROW_H1:ROW_H1 + H, c1] += s * W_hh1[r, :].T
            WG[ROW_ONES, c1] += s * b1[r]
            WG[ROW_X0 + xslot, c1] += s * W_ih1[r, 0]
        if l2:
            c2 = slice(col0 + ROW_H2, col0 + ROW_H2 + H)  # rows 64..114
            WG[ROW_H1:ROW_H1 + H, c2] += s * W_ih2[r, :].T
            WG[ROW_H2:ROW_H2 + H, c2] += s * W_hh2[r, :].T
            WG[ROW_ONES, c2] += s * b2[r]

    for slot in range(NXSLOT):
        for bi, gate in enumerate(order):
            fill_bank((slot * 4 + bi) * MW, gate, True, True, slot)
    for bi, gate in enumerate(order):
        fill_bank((L1B + bi) * MW, gate, True, False, 0)
        fill_bank((L2B + bi) * MW, gate, False, True, 0)
    WG[ROW_H2:ROW_H2 + H, HEADCOL] = W_lin[0, :]
    WG[ROW_ONES, HEADCOL] = float(np.asarray(b_lin).reshape(-1)[0])
    return np.ascontiguousarray(WG).astype(np.float32)


def build_core_kernel(T, B, groups=2, use_f32r=True):
    """Per-core Bass kernel. Inputs: xT [T+1, B] (row 0 = ones), WG.
    Output: out_bt [B, T]."""
    import concourse.bacc as bacc
    import concourse.mybir as mybir
    from concourse.tile import TileContext

    fp = mybir.dt.float32
    fpr = mybir.dt.float32r if use_f32r else fp
    Bg = B // groups
    assert B % groups == 0 and Bg % 128 == 0

    nc = bacc.Bacc("TRN2", target_bir_lowering=False, debug=False)
    xT = nc.dram_tensor("xT", [T + 1, B], fpr, kind="ExternalInput")
    WG = nc.dram_tensor("WG", [K_STK, NCOL], fpr, kind="ExternalInput")
    out_bt = nc.dram_tensor("out_bt", [B, T], fp, kind="ExternalOutput")

    C = min(128, T)          # head columns buffered in PSUM between flushes
    assert T % C == 0
    nchunk = B // 128
    assert T % XBANK == 0

    add = mybir.AluOpType.add
    mult = mybir.AluOpType.mult
    tanh = mybir.ActivationFunctionType.Tanh
    sigmoid = mybir.ActivationFunctionType.Sigmoid
    f_sig = CONFIG["f_sig"]
    gfunc = {"i": tanh, "g": tanh,
             "f": sigmoid if f_sig else tanh, "o": sigmoid}
    eng = {"dve": nc.vector, "pool": nc.gpsimd}
    v_eng = eng[CONFIG["v_eng"]]
    u_eng = eng[CONFIG["u_eng"]]
    ct_eng = eng[CONFIG["ct_eng"]]
    tails_inline = CONFIG["tails_inline"]
    ht_split = CONFIG["ht_eng"] == "split"
    ht_eng = None if ht_split else eng[CONFIG["ht_eng"]]
    tiles_cfg = [(CONFIG["t1"], CONFIG["t1_out"], CONFIG["t1_split"]),
                 (CONFIG["t2"], CONFIG["t2_out"], CONFIG["t2_split"])]
    order = _gate_order()
    gate_tile = {}
    for tix, (gates, out, split) in enumerate(tiles_cfg):
        for ci, gname in enumerate(gates):
            gate_tile[gname] = (tix, ci)
    sb_gate = {g: tiles_cfg[gate_tile[g][0]][1] == "sb" for g in "igfo"}
    # legality: pool needs SBUF operands and supports only tensor_tensor
    # (no scalar_tensor_tensor); others: <=1 PSUM input per instruction
    assert sb_gate["i"] or sb_gate["g"], "v would read 2 PSUM inputs"
    assert CONFIG["v_eng"] != "pool", "v is stt; unsupported on Pool"
    if CONFIG["u_eng"] == "pool":
        assert f_sig and sb_gate["f"]
    if CONFIG["ct_eng"] == "pool":
        assert f_sig
    if CONFIG["ht_eng"] in ("pool", "split"):
        assert sb_gate["o"]

    with TileContext(nc) as tc:
        with (
            tc.tile_pool(name="persist", bufs=1) as persist,
            tc.tile_pool(name="gpsum", bufs=1, space="PSUM") as gpsum,
            tc.tile_pool(name="opsum", bufs=1, space="PSUM") as opsum,
        ):
            wg = persist.tile([K_STK, NCOL], fpr)
            nc.sync.dma_start(out=wg, in_=WG[:, :])

            stk = persist.tile([K_STK, B], fpr, tag="stk")
            ctt = persist.tile([GP, B], fp, tag="ctt")
            nc.vector.memset(stk[:, :].bitcast(fp), 0.0)
            nc.vector.memset(ctt[:, :], 0.0)
            nc.sync.dma_start(out=stk[ROW_ONES:ROW_ONES + 1, :],
                              in_=xT[0:1, :])
            # preload x windows 0 and 4 (phases 0..7)
            nc.sync.dma_start(out=stk[ROW_X0:ROW_X0 + XBANK, :],
                              in_=xT[1:1 + XBANK, :])
            if T > XBANK:
                nc.sync.dma_start(
                    out=stk[ROW_X0 + XBANK:ROW_X0 + 2 * XBANK, :],
                    in_=xT[1 + XBANK:1 + 2 * XBANK, :])

            gpt = [[gpsum.tile([GP, 2 * Bg], fp, tag=f"gp{t}{g}",
                               name=f"gp{t}{g}") for t in range(2)]
                   for g in range(groups)]
            sbt = [[persist.tile([GP, 2 * Bg], fp, tag=f"sb{t}{g}",
                                 name=f"sb{t}{g}")
                    if tiles_cfg[t][1] == "sb" else None for t in range(2)]
                   for g in range(groups)]
            pos = opsum.tile([128, nchunk * C], fp, tag="pos")
            us = [persist.tile([GP, Bg], fp, tag=f"u{g}", name=f"u{g}")
                  for g in range(groups)]
            vs = [persist.tile([GP, Bg], fp, tag=f"v{g}", name=f"v{g}")
                  for g in range(groups)]
            tcls = [persist.tile([GP, Bg], fp, tag=f"tcl{g}", name=f"tcl{g}")
                    for g in range(groups)]
            ost = persist.tile([128, nchunk * C], fp, tag="ost", name="ost")

            def gate_res(g, gname):
                tix, ci = gate_tile[gname]
                src = sbt[g][tix] if tiles_cfg[tix][1] == "sb" else gpt[g][tix]
                return src[:, ci * Bg:(ci + 1) * Bg]

            def emit_tail(g):
                cols = slice(g * Bg, (g + 1) * Bg)
                # tcl = tanh(ct/2) = tanh(c)
                nc.scalar.activation(tcls[g], ctt[:, cols], tanh, scale=0.5)
                # ht = so * tcl  (h stored undoubled; so = sigmoid)
                so = gate_res(g, "o")
                if ht_split:
                    hd = CONFIG["ht_dve_cols"]
                    c0 = g * Bg
                    nc.vector.tensor_tensor(
                        stk[ROW_H1:ROW_H1 + GP, c0:c0 + hd],
                        so[:, 0:hd], tcls[g][:, 0:hd], mult)
                    nc.gpsimd.tensor_tensor(
                        stk[ROW_H1:ROW_H1 + GP, c0 + hd:c0 + Bg],
                        so[:, hd:Bg], tcls[g][:, hd:Bg], mult)
                else:
                    ht_eng.tensor_tensor(
                        stk[ROW_H1:ROW_H1 + GP, cols], so, tcls[g], mult)

            for q in range(T + 2):
                # ---- x prefetch: window w = q+6 (covers phases w..w+3),
                # rotating over 3 banks of 4 partition rows
                w = q + 6
                if w % XBANK == 0 and w < T:
                    bank = (w // XBANK) % (NXSLOT // XBANK)
                    nc.sync.dma_start(
                        out=stk[ROW_X0 + bank * XBANK:
                                ROW_X0 + (bank + 1) * XBANK, :],
                        in_=xT[1 + w:1 + w + XBANK, :])

                for g in range(groups):
                    cols = slice(g * Bg, (g + 1) * Bg)
                    if q > T:
                        continue
                    if q == 0:
                        vb = L1B
                    elif q == T:
                        vb = L2B
                    else:
                        vb = (q % NXSLOT) * 4
                    rhs = stk[0:K_STK, cols]
                    for bi in range(4):
                        tix, ci = gate_tile[order[bi]]
                        nc.tensor.matmul(
                            gpt[g][tix][0:GP, ci * Bg:(ci + 1) * Bg],
                            wg[0:K_STK, (vb + bi) * MW:(vb + bi + 1) * MW],
                            rhs, start=True, stop=True)
                    # tanh per tile (joint or split), v as soon as i,g are
                    # done, u as soon as f is done
                    done = set()
                    emitted_v = emitted_u = False
                    for tix, (gates, out, split) in enumerate(tiles_cfg):
                        src = gpt[g][tix]
                        dst = sbt[g][tix] if out == "sb" else src
                        if split:
                            for ci, gname in enumerate(gates):
                                nc.scalar.activation(
                                    dst[:, ci * Bg:(ci + 1) * Bg],
                                    src[:, ci * Bg:(ci + 1) * Bg],
                                    gfunc[gname])
                                done.add(gname)
                                emitted_v, emitted_u = _maybe_uv(
                                    nc, g, cols, done, emitted_v, emitted_u,
                                    v_eng, u_eng, vs, us, ctt, gate_res,
                                    add, mult, f_sig)
                        else:
                            # one func per instruction
                            assert len({gfunc[x] for x in gates}) == 1
                            nc.scalar.activation(dst[:, :], src[:, :],
                                                 gfunc[gates[0]])
                            done.update(gates)
                            emitted_v, emitted_u = _maybe_uv(
                                nc, g, cols, done, emitted_v, emitted_u,
                                v_eng, u_eng, vs, us, ctt, gate_res,
                                add, mult, f_sig)
                    if f_sig:
                        # ct' = u + v  (u = sf*ct)
                        ct_eng.tensor_tensor(ctt[:, cols], us[g], vs[g], add)
                    else:
                        # ct' = 0.5*u + v  (u = (tf+1)*ct)
                        ct_eng.scalar_tensor_tensor(
                            ctt[:, cols], us[g], 0.5, vs[g], mult, add)
                    if tails_inline:
                        emit_tail(g)

                # ---- head (reads stk h2; with inline tails this
                # phase's ht is already in stk, so the head step shifts)
                hq0 = 1 if tails_inline else 2
                if q >= hq0 and q - hq0 < T:
                    t = q - hq0
                    tcc = t % C
                    for k in range(nchunk):
                        nc.tensor.matmul(
                            pos[:, k * C + tcc:k * C + tcc + 1],
                            stk[64:116, k * 128:(k + 1) * 128].bitcast(fp),
                            wg[64:116, HEADCOL:HEADCOL + 1].bitcast(fp),
                            start=True, stop=True)
                    if tcc == C - 1:
                        t0 = t - (C - 1)
                        for k in range(nchunk):
                            # stage PSUM->SBUF on DVE (DMA and Pool can't
                            # read PSUM)
                            nc.vector.tensor_scalar(
                                ost[:, k * C:(k + 1) * C],
                                pos[:, k * C:(k + 1) * C], 0.0, None, add)
                            nc.sync.dma_start(
                                out=out_bt[k * 128:(k + 1) * 128, t0:t0 + C],
                                in_=ost[:, k * C:(k + 1) * C])

                # ---- tails (shared loop unless inlined per group)
                if q <= T and not tails_inline:
                    for g in range(groups):
                        emit_tail(g)
    nc.compile()
    return nc


def _maybe_uv(nc, g, cols, done, emitted_v, emitted_u, v_eng, u_eng,
              vs, us, ctt, gate_res, add, mult, f_sig):
    if not emitted_v and "i" in done and "g" in done:
        # v = (ti+1)*tg  (= 2*sigma_i*gtilde)
        v_eng.scalar_tensor_tensor(
            vs[g], gate_res(g, "i"), 1.0, gate_res(g, "g"), add, mult)
        emitted_v = True
    if not emitted_u and "f" in done:
        if f_sig:
            # u = sf * ct  (= 2*sigma_f*c)
            u_eng.tensor_tensor(us[g], gate_res(g, "f"), ctt[:, cols], mult)
        else:
            # u = (tf+1)*ct
            u_eng.scalar_tensor_tensor(
                us[g], gate_res(g, "f"), 1.0, ctt[:, cols], add, mult)
        emitted_u = True
    return emitted_v, emitted_u


_NC_CACHE = {}


def _get_nc(T, B, groups=2, use_f32r=True):
    key = (T, B, groups, use_f32r, tuple(sorted(CONFIG.items())))
    if key not in _NC_CACHE:
        _NC_CACHE[key] = build_core_kernel(T, B, groups, use_f32r)
    return _NC_CACHE[key]


def kernel(input, W_ih1, W_hh1, b_ih1, b_hh1, W_ih2, W_hh2, b_ih2, b_hh2,
           W_lin, b_lin, _groups=2, _use_f32r=True):
    from concourse import bass_utils

    input = np.asarray(input, dtype=np.float32)
    B, T = input.shape
    Bc = B // N_CORES
    WG = _build_weights(
        np.asarray(W_ih1, np.float64), np.asarray(W_hh1, np.float64),
        np.asarray(b_ih1, np.float64), np.asarray(b_hh1, np.float64),
        np.asarray(W_ih2, np.float64), np.asarray(W_hh2, np.float64),
        np.asarray(b_ih2, np.float64), np.asarray(b_hh2, np.float64),
        np.asarray(W_lin, np.float64), np.asarray(b_lin, np.float64))
    xT = np.concatenate([np.ones((1, B), np.float32),
                         input.T.astype(np.float32)])
    nc = _get_nc(T, Bc, _groups, _use_f32r)
    in_maps = [
        {"xT": np.ascontiguousarray(xT[:, c * Bc:(c + 1) * Bc]), "WG": WG}
        for c in range(N_CORES)
    ]
    res = bass_utils.run_bass_kernel_spmd(
        nc, in_maps, core_ids=list(range(N_CORES)), trace=False)
    outs = [res.results[c]["out_bt"] for c in range(N_CORES)]
    out = np.concatenate(outs, axis=0)
    return out.astype(np.float32)


# revision 51
# speedup vs baseline: 1.2299x; 1.0029x over previous
"""Two-layer LSTM (H=51) over [B=4096, T=256] on 8 NeuronCores.

Data-parallel over batch (512 per core), skewed software pipeline over T+2
phases: phase q computes layer-1 of step q together with layer-2 of step
q-1 (merged into the same matmuls), and the linear head of step q-2.

Per phase (per batch-group of 256):
  - 4 merged gate matmuls: one lhsT carries BOTH layers' weights
    (layer-1 -> output rows 0..50, layer-2 -> rows 64..114), two gate
    banks per PSUM tile (assignment configurable).
  - x is prefetched 4 steps per DMA into 12 rotating rhs partition slots
    (rows 116..127); each of the 12 lhsT variants reads its own x row.
  - sigma(z) = (tanh(z/2)+1)/2 with the 1/2 folded into the weights and
    states stored doubled (ht=2h, ct=2c); elementwise tail:
    u=(tf+1)*ct, v=(ti+1)*tg, ct'=0.5u+v, tcl=tanh(ct'/2), ht=(to+1)*tcl.

Hardware legality notes (bir verifier):
  - Pool/gpsimd instructions cannot access PSUM at all.
  - Other engines may read at most ONE non-scalar input from PSUM.
"""

import numpy as np

H = 51
T_FULL = 256
B_FULL = 4096
N_CORES = 8

ROW_H1 = 0      # rows 0..50: ht1 (= 2*h1)
ROW_H2 = 64     # rows 64..114: ht2 (= 2*h2)  (64: 32-aligned lhsT base for head)
ROW_ONES = 115  # bias row
ROW_X0 = 116    # rows 116..127: 12 x slots (3 rotating DMA banks of 4)
NXSLOT = 12
XBANK = 4       # steps per x-prefetch DMA
K_STK = 128
GP = 115        # gate-row space: 0..50 layer1, 51..63 junk, 64..114 layer2
MW = 115        # matmul output width per bank

N_VAR = NXSLOT * 4          # merged variant banks
L1B = N_VAR                 # 4 layer-1-only banks (phase 0, x slot 0)
L2B = N_VAR + 4             # 4 layer-2-only banks (phase T)
HEADCOL = (N_VAR + 8) * MW  # head column
NCOL = HEADCOL + 1

# schedule/assignment knobs (sweepable):
#   t1/t2: gate pair per PSUM tile; t?_out: tanh dest ("sb" SBUF tile,
#   "ip" in-place PSUM); t?_split: one tanh instr per gate instead of a
#   joint one; engines: "dve" | "pool" (pool requires SBUF operands).
CONFIG = {
    "t1": "ig", "t2": "fo", "t1_out": "sb", "t2_out": "sb",
    "t1_split": False, "t2_split": True,
    "v_eng": "dve", "u_eng": "dve", "ct_eng": "dve", "ht_eng": "dve",
    # f_sig: f-gate is a real Sigmoid (joint [f,o] sigmoid instr);
    # u = sf*ct and ct' = u+v become plain tensor_tensor ops.
    "f_sig": True,
    # ht_eng "split": ht computed in two column-halves in parallel,
    # cols [0:ht_dve_cols] on DVE and the rest on Pool.
    "ht_dve_cols": 176,
    # issue tcl/ht inside each group's block instead of a shared tail loop
    "tails_inline": True,
    # defer the ht ops of both groups to the end of the phase (tcl stays
    # inline) so a waiting ht doesn't head-of-line-block the next group's
    # DVE work
    "ht_defer": True,
}


def _gate_order():
    return list(CONFIG["t1"]) + list(CONFIG["t2"])


def _build_weights(W_ih1, W_hh1, b_ih1, b_hh1, W_ih2, W_hh2, b_ih2, b_hh2,
                   W_lin, b_lin):
    """Host-side packing of lhsT weight banks -> WG [K_STK, NCOL] f32."""
    b1 = (b_ih1 + b_hh1).astype(np.float64)
    b2 = (b_ih2 + b_hh2).astype(np.float64)
    idx = {"i": np.arange(0, H), "f": np.arange(H, 2 * H),
           "g": np.arange(2 * H, 3 * H), "o": np.arange(3 * H, 4 * H)}
    WG = np.zeros((K_STK, NCOL), dtype=np.float64)
    order = _gate_order()

    f_sig = CONFIG["f_sig"]

    def fill_bank(col0, gate, l1, l2, xslot):
        # h is stored UNdoubled; c doubled (ct=2c). i (and f unless
        # f_sig) use the tanh-half-angle trick (s=0.5); g is tanh;
        # o (and f if f_sig) are real Sigmoids.
        r = idx[gate]
        s = 0.5 if (gate == "i" or (gate == "f" and not f_sig)) else 1.0
        if l1:
            c1 = slice(col0, col0 + H)  # output rows 0..50
            WG[ROW_H1:ROW_H1 + H, c1] += s * W_hh1[r, :].T
            WG[ROW_ONES, c1] += s * b1[r]
            WG[ROW_X0 + xslot, c1] += s * W_ih1[r, 0]
        if l2:
            c2 = slice(col0 + ROW_H2, col0 + ROW_H2 + H)  # rows 64..114
            WG[ROW_H1:ROW_H1 + H, c2] += s * W_ih2[r, :].T
            WG[ROW_H2:ROW_H2 + H, c2] += s * W_hh2[r, :].T
            WG[ROW_ONES, c2] += s * b2[r]

    for slot in range(NXSLOT):
        for bi, gate in enumerate(order):
            fill_bank((slot * 4 + bi) * MW, gate, True, True, slot)
    for bi, gate in enumerate(order):
        fill_bank((L1B + bi) * MW, gate, True, False, 0)
        fill_bank((L2B + bi) * MW, gate, False, True, 0)
    WG[ROW_H2:ROW_H2 + H, HEADCOL] = W_lin[0, :]
    WG[ROW_ONES, HEADCOL] = float(np.asarray(b_lin).reshape(-1)[0])
    return np.ascontiguousarray(WG).astype(np.float32)


def build_core_kernel(T, B, groups=2, use_f32r=True):
    """Per-core Bass kernel. Inputs: xT [T+1, B] (row 0 = ones), WG.
    Output: out_bt [B, T]."""
    import concourse.bacc as bacc
    import concourse.mybir as mybir
    from concourse.tile import TileContext

    fp = mybir.dt.float32
    fpr = mybir.dt.float32r if use_f32r else fp
    Bg = B // groups
    assert B % groups == 0 and Bg % 128 == 0

    nc = bacc.Bacc("TRN2", target_bir_lowering=False, debug=False)
    xT = nc.dram_tensor("xT", [T + 1, B], fpr, kind="ExternalInput")
    WG = nc.dram_tensor("WG", [K_STK, NCOL], fpr, kind="ExternalInput")
    out_bt = nc.dram_tensor("out_bt", [B, T], fp, kind="ExternalOutput")

    C = min(128, T)          # head columns buffered in PSUM between flushes
    assert T % C == 0
    nchunk = B // 128
    assert T % XBANK == 0

    add = mybir.AluOpType.add
    mult = mybir.AluOpType.mult
    tanh = mybir.ActivationFunctionType.Tanh
    sigmoid = mybir.ActivationFunctionType.Sigmoid
    f_sig = CONFIG["f_sig"]
    gfunc = {"i": tanh, "g": tanh,
             "f": sigmoid if f_sig else tanh, "o": sigmoid}
    eng = {"dve": nc.vector, "pool": nc.gpsimd}
    v_eng = eng[CONFIG["v_eng"]]
    u_eng = eng[CONFIG["u_eng"]]
    ct_eng = eng[CONFIG["ct_eng"]]
    tails_inline = CONFIG["tails_inline"]
    ht_split = CONFIG["ht_eng"] == "split"
    ht_eng = None if ht_split else eng[CONFIG["ht_eng"]]
    tiles_cfg = [(CONFIG["t1"], CONFIG["t1_out"], CONFIG["t1_split"]),
                 (CONFIG["t2"], CONFIG["t2_out"], CONFIG["t2_split"])]
    order = _gate_order()
    gate_tile = {}
    for tix, (gates, out, split) in enumerate(tiles_cfg):
        for ci, gname in enumerate(gates):
            gate_tile[gname] = (tix, ci)
    sb_gate = {g: tiles_cfg[gate_tile[g][0]][1] == "sb" for g in "igfo"}
    # legality: pool needs SBUF operands and supports only tensor_tensor
    # (no scalar_tensor_tensor); others: <=1 PSUM input per instruction
    assert sb_gate["i"] or sb_gate["g"], "v would read 2 PSUM inputs"
    assert CONFIG["v_eng"] != "pool", "v is stt; unsupported on Pool"
    if CONFIG["u_eng"] == "pool":
        assert f_sig and sb_gate["f"]
    if CONFIG["ct_eng"] == "pool":
        assert f_sig
    if CONFIG["ht_eng"] in ("pool", "split"):
        assert sb_gate["o"]

    with TileContext(nc) as tc:
        with (
            tc.tile_pool(name="persist", bufs=1) as persist,
            tc.tile_pool(name="gpsum", bufs=1, space="PSUM") as gpsum,
            tc.tile_pool(name="opsum", bufs=1, space="PSUM") as opsum,
        ):
            wg = persist.tile([K_STK, NCOL], fpr)
            # boundary banks + head column first (phase 0 needs them),
            # then the variant banks in parallel chunks
            c_bnd = L1B * MW
            nc.sync.dma_start(out=wg[:, c_bnd:NCOL], in_=WG[:, c_bnd:NCOL])
            nvar4 = (c_bnd // 4) // MW * MW
            for j in range(4):
                c0, c1 = j * nvar4, (j + 1) * nvar4 if j < 3 else c_bnd
                nc.sync.dma_start(out=wg[:, c0:c1], in_=WG[:, c0:c1])

            stk = persist.tile([K_STK, B], fpr, tag="stk")
            ctt = persist.tile([GP, B], fp, tag="ctt")
            nc.vector.memset(stk[:, :].bitcast(fp), 0.0)
            nc.vector.memset(ctt[:, :], 0.0)
            nc.sync.dma_start(out=stk[ROW_ONES:ROW_ONES + 1, :],
                              in_=xT[0:1, :])
            # preload x windows 0 and 4 (phases 0..7)
            nc.sync.dma_start(out=stk[ROW_X0:ROW_X0 + XBANK, :],
                              in_=xT[1:1 + XBANK, :])
            if T > XBANK:
                nc.sync.dma_start(
                    out=stk[ROW_X0 + XBANK:ROW_X0 + 2 * XBANK, :],
                    in_=xT[1 + XBANK:1 + 2 * XBANK, :])

            gpt = [[gpsum.tile([GP, 2 * Bg], fp, tag=f"gp{t}{g}",
                               name=f"gp{t}{g}") for t in range(2)]
                   for g in range(groups)]
            sbt = [[persist.tile([GP, 2 * Bg], fp, tag=f"sb{t}{g}",
                                 name=f"sb{t}{g}")
                    if tiles_cfg[t][1] == "sb" else None for t in range(2)]
                   for g in range(groups)]
            pos = opsum.tile([128, nchunk * C], fp, tag="pos")
            us = [persist.tile([GP, Bg], fp, tag=f"u{g}", name=f"u{g}")
                  for g in range(groups)]
            vs = [persist.tile([GP, Bg], fp, tag=f"v{g}", name=f"v{g}")
                  for g in range(groups)]
            tcls = [persist.tile([GP, Bg], fp, tag=f"tcl{g}", name=f"tcl{g}")
                    for g in range(groups)]
            ost = persist.tile([128, nchunk * C], fp, tag="ost", name="ost")

            def gate_res(g, gname):
                tix, ci = gate_tile[gname]
                src = sbt[g][tix] if tiles_cfg[tix][1] == "sb" else gpt[g][tix]
                return src[:, ci * Bg:(ci + 1) * Bg]

            def emit_tail(g, part="both"):
                cols = slice(g * Bg, (g + 1) * Bg)
                if part in ("both", "tcl"):
                    # tcl = tanh(ct/2) = tanh(c)
                    nc.scalar.activation(tcls[g], ctt[:, cols], tanh,
                                         scale=0.5)
                if part == "tcl":
                    return
                # ht = so * tcl  (h stored undoubled; so = sigmoid)
                so = gate_res(g, "o")
                if ht_split:
                    hd = CONFIG["ht_dve_cols"]
                    c0 = g * Bg
                    nc.vector.tensor_tensor(
                        stk[ROW_H1:ROW_H1 + GP, c0:c0 + hd],
                        so[:, 0:hd], tcls[g][:, 0:hd], mult)
                    nc.gpsimd.tensor_tensor(
                        stk[ROW_H1:ROW_H1 + GP, c0 + hd:c0 + Bg],
                        so[:, hd:Bg], tcls[g][:, hd:Bg], mult)
                else:
                    ht_eng.tensor_tensor(
                        stk[ROW_H1:ROW_H1 + GP, cols], so, tcls[g], mult)

            for q in range(T + 2):
                # ---- x prefetch: window w = q+6 (covers phases w..w+3),
                # rotating over 3 banks of 4 partition rows
                w = q + 6
                if w % XBANK == 0 and w < T:
                    bank = (w // XBANK) % (NXSLOT // XBANK)
                    nc.sync.dma_start(
                        out=stk[ROW_X0 + bank * XBANK:
                                ROW_X0 + (bank + 1) * XBANK, :],
                        in_=xT[1 + w:1 + w + XBANK, :])

                for g in range(groups):
                    cols = slice(g * Bg, (g + 1) * Bg)
                    if q > T:
                        continue
                    if q == 0:
                        vb = L1B
                    elif q == T:
                        vb = L2B
                    else:
                        vb = (q % NXSLOT) * 4
                    rhs = stk[0:K_STK, cols]
                    for bi in range(4):
                        tix, ci = gate_tile[order[bi]]
                        nc.tensor.matmul(
                            gpt[g][tix][0:GP, ci * Bg:(ci + 1) * Bg],
                            wg[0:K_STK, (vb + bi) * MW:(vb + bi + 1) * MW],
                            rhs, start=True, stop=True)
                    # tanh per tile (joint or split), v as soon as i,g are
                    # done, u as soon as f is done
                    done = set()
                    emitted_v = emitted_u = False
                    for tix, (gates, out, split) in enumerate(tiles_cfg):
                        src = gpt[g][tix]
                        dst = sbt[g][tix] if out == "sb" else src
                        if split:
                            for ci, gname in enumerate(gates):
                                nc.scalar.activation(
                                    dst[:, ci * Bg:(ci + 1) * Bg],
                                    src[:, ci * Bg:(ci + 1) * Bg],
                                    gfunc[gname])
                                done.add(gname)
                                emitted_v, emitted_u = _maybe_uv(
                                    nc, g, cols, done, emitted_v, emitted_u,
                                    v_eng, u_eng, vs, us, ctt, gate_res,
                                    add, mult, f_sig)
                        else:
                            # one func per instruction
                            assert len({gfunc[x] for x in gates}) == 1
                            nc.scalar.activation(dst[:, :], src[:, :],
                                                 gfunc[gates[0]])
                            done.update(gates)
                            emitted_v, emitted_u = _maybe_uv(
                                nc, g, cols, done, emitted_v, emitted_u,
                                v_eng, u_eng, vs, us, ctt, gate_res,
                                add, mult, f_sig)
                    if f_sig:
                        # ct' = u + v  (u = sf*ct)
                        ct_eng.tensor_tensor(ctt[:, cols], us[g], vs[g], add)
                    else:
                        # ct' = 0.5*u + v  (u = (tf+1)*ct)
                        ct_eng.scalar_tensor_tensor(
                            ctt[:, cols], us[g], 0.5, vs[g], mult, add)
                    if tails_inline:
                        emit_tail(g, "tcl" if CONFIG["ht_defer"] else "both")

                # ---- head (reads stk h2; with inline tails this
                # phase's ht is already in stk, so the head step shifts)
                hq0 = 1 if tails_inline else 2
                if q >= hq0 and q - hq0 < T:
                    t = q - hq0
                    tcc = t % C
                    for k in range(nchunk):
                        nc.tensor.matmul(
                            pos[:, k * C + tcc:k * C + tcc + 1],
                            stk[64:116, k * 128:(k + 1) * 128].bitcast(fp),
                            wg[64:116, HEADCOL:HEADCOL + 1].bitcast(fp),
                            start=True, stop=True)
                    if tcc == C - 1:
                        t0 = t - (C - 1)
                        for k in range(nchunk):
                            # stage PSUM->SBUF on DVE (DMA and Pool can't
                            # read PSUM)
                            nc.vector.tensor_scalar(
                                ost[:, k * C:(k + 1) * C],
                                pos[:, k * C:(k + 1) * C], 0.0, None, add)
                            nc.sync.dma_start(
                                out=out_bt[k * 128:(k + 1) * 128, t0:t0 + C],
                                in_=ost[:, k * C:(k + 1) * C])

                # ---- tails (shared loop unless inlined per group)
                if q <= T and not tails_inline:
                    for g in range(groups):
                        emit_tail(g)
                if q <= T and tails_inline and CONFIG["ht_defer"]:
                    for g in range(groups):
                        emit_tail(g, "ht")
    nc.compile()
    return nc


def _maybe_uv(nc, g, cols, done, emitted_v, emitted_u, v_eng, u_eng,
              vs, us, ctt, gate_res, add, mult, f_sig):
    if not emitted_v and "i" in done and "g" in done:
        # v = (ti+1)*tg  (= 2*sigma_i*gtilde)
        v_eng.scalar_tensor_tensor(
            vs[g], gate_res(g, "i"), 1.0, gate_res(g, "g"), add, mult)
        emitted_v = True
    if not emitted_u and "f" in done:
        if f_sig:
            # u = sf * ct  (= 2*sigma_f*c)
            u_eng.tensor_tensor(us[g], gate_res(g, "f"), ctt[:, cols], mult)
        else:
            # u = (tf+1)*ct
            u_eng.scalar_tensor_tensor(
                us[g], gate_res(g, "f"), 1.0, ctt[:, cols], add, mult)
        emitted_u = True
    return emitted_v, emitted_u


_NC_CACHE = {}


def _get_nc(T, B, groups=2, use_f32r=True):
    key = (T, B, groups, use_f32r, tuple(sorted(CONFIG.items())))
    if key not in _NC_CACHE:
        _NC_CACHE[key] = build_core_kernel(T, B, groups, use_f32r)
    return _NC_CACHE[key]


def kernel(input, W_ih1, W_hh1, b_ih1, b_hh1, W_ih2, W_hh2, b_ih2, b_hh2,
           W_lin, b_lin, _groups=2, _use_f32r=True):
    from concourse import bass_utils

    input = np.asarray(input, dtype=np.float32)
    B, T = input.shape
    Bc = B // N_CORES
    WG = _build_weights(
        np.asarray(W_ih1, np.float64), np.asarray(W_hh1, np.float64),
        np.asarray(b_ih1, np.float64), np.asarray(b_hh1, np.float64),
        np.asarray(W_ih2, np.float64), np.asarray(W_hh2, np.float64),
        np.asarray(b_ih2, np.float64), np.asarray(b_hh2, np.float64),
        np.asarray(W_lin, np.float64), np.asarray(b_lin, np.float64))
    xT = np.concatenate([np.ones((1, B), np.float32),
                         input.T.astype(np.float32)])
    nc = _get_nc(T, Bc, _groups, _use_f32r)
    in_maps = [
        {"xT": np.ascontiguousarray(xT[:, c * Bc:(c + 1) * Bc]), "WG": WG}
        for c in range(N_CORES)
    ]
    res = bass_utils.run_bass_kernel_spmd(
        nc, in_maps, core_ids=list(range(N_CORES)), trace=False)
    outs = [res.results[c]["out_bt"] for c in range(N_CORES)]
    out = np.concatenate(outs, axis=0)
    return out.astype(np.float32)
